# revision 10
# baseline (speedup 1.0000x reference)
"""HGATConv v3: head-per-core + sorted staircase + fp8 masks.

Sharding: core c = (head h=c//2, half=c%2). Core owns the 2048 output rows
at stride-2 positions of the si_h-sorted order (so every core's i-tile t
covers the same si-quantile band -> one SPMD program works for all cores).
j (all 4096) is sorted by sj_h ascending per head.

Scores si/sj are host-computed (rank-8 GEMM). With s=si+sj and e^{si}
divided out of the softmax, the edge weight is
    w = max(e^{-0.8 si} * e^{0.2 sj}, e^{sj}) = max(E21[i]*v[j], u[j])
and the branch boundary sj = -si is MONOTONE in the sorted orders: the
16x32 (i-tile x j-block) grid splits into pure-A (s<0), pure-B (s>=0) and a
thin mixed staircase band (host-computed union across cores, baked into the
compiled program; cache keyed on it).

Per DR-step d (256 j's, fp8 DoubleRow):
  PE:   ph = x_jj @ W_h (4 fp8xbf16 matmuls, 128 cols)
        per i-tile t: acc[t](+)= lhsT.T @ rhs2  where
          t <  a[jj]: lhsT = maskHYB (fp8 = mask*E21, host-folded), rhs2A=v*[h|1]
          t >= bs[jj]: lhsT = maskHYB (fp8 = raw mask),             rhs2B=u*[h|1]
          else mixed: lhsT = pm (bf16, DVE: cast + max(E21*g,1)*mask), rhs2B
        (mixed-dtype fp8 lhsT x bf16 rhs matmuls verified on HW)
  ACT:  rhs2A/rhs2B psum->sbuf casts with per-partition scale v/u
  DVE:  only the thin mixed band (cast fp8->bf16, TS, TT) + aug copy
The 129th (aug) rhs column carries v/u -> denominators accumulate free.
PSUM: 16 aug-tiles [128,129] packed 3-per-bank (6 banks) + 2 ph banks.
DMA: maskHYB 8.4MB fp8 + xT 2MB fp8 + out 1MB; mask on SP queue, x on PE
queue, out on ACT queue (keeps each sequencer under ~1 DMA/block).
"""

import sys
import numpy as np

if "/opt/trn_rl_repo" not in sys.path:
    sys.path.insert(0, "/opt/trn_rl_repo")

H, D = 4, 128
N, F = 4096, 512
M = 8
NI = 2048              # i rows per core (one head, half the nodes)
TI = NI // 128         # 16 i tiles
JB = N // 128          # 32 j blocks
KB = F // 128          # 4 contraction tiles
AUG = D + 1            # 129
ALPHA = 0.2

_CACHE = {}


def _build_nc(aArr, bsArr):
    import concourse.bacc as bacc
    from concourse import mybir
    from concourse.tile import TileContext

    f32 = mybir.dt.float32
    bf16 = mybir.dt.bfloat16
    fp8 = mybir.dt.float8e4
    Alu = mybir.AluOpType
    Act = mybir.ActivationFunctionType

    nc = bacc.Bacc(num_swdge_queues=4)
    xT_d = nc.declare_dram_parameter("xT", [128, N * KB], bf16, isOutput=False)
    Wh_d = nc.declare_dram_parameter("Wh", [F, D], bf16, isOutput=False)
    mk_d = nc.declare_dram_parameter("maskH", [N // 2, 2 * NI], fp8, isOutput=False)
    E21_d = nc.declare_dram_parameter("E21", [128, NI], bf16, isOutput=False)
    PJ_d = nc.declare_dram_parameter("PJ", [128, JB * 3], f32, isOutput=False)
    UV_d = nc.declare_dram_parameter("UV", [128, JB * 2], bf16, isOutput=False)
    mm_d = nc.declare_dram_parameter("maskM", [N // 2, 2 * 384], bf16,
                                     isOutput=False)
    out_d = nc.declare_dram_parameter("out", [NI, D], bf16, isOutput=True)

    Wh_v = Wh_d.rearrange("(t p) d -> p t d", p=128)

    with TileContext(nc) as tc:
        with tc.tile_pool(name="const", bufs=1) as cpool:
            Wh_sb = cpool.tile([128, KB, D], bf16)
            E21 = cpool.tile([128, NI], bf16)
            PJs = cpool.tile([128, JB, 3], f32)
            UVs = cpool.tile([128, JB, 2], bf16)
            nc.gpsimd.dma_start(Wh_sb[:], Wh_v[:])

            with (
                tc.tile_pool(name="accp", bufs=1, space="PSUM") as accp,
                tc.tile_pool(name="php", bufs=2, space="PSUM") as php,
                tc.tile_pool(name="stream", bufs=4) as stream,
                tc.tile_pool(name="pp", bufs=2) as pp,
            ):
                accb = [accp.tile([128, 512], f32, name=f"accb{i}")
                        for i in range(6)]

                def accv(t):
                    # DR psum writes must be 8B-aligned: 130-stride slots
                    return accb[t // 3][:, (t % 3) * 130:(t % 3) * 130 + AUG]

                DB = JB // 2
                maxw = max(128, 128 * max(b - a for a, b in zip(aArr, bsArr)))
                ph_t = [None, None]
                mk_t = [None] * (JB // 2)
                rhsA_t = [None] * (JB // 2)
                rhsB_t = [None] * (JB // 2)
                pm_t = [None] * (JB // 2)

                def issue_mask(d):
                    mk = stream.tile([128, 2 * NI], fp8, tag="mk")
                    nc.sync.dma_start(mk[:, 0:NI],
                                      mk_d[d * 128:(d + 1) * 128, 0:NI])
                    nc.sync.dma_start(mk[:, NI:2 * NI],
                                      mk_d[d * 128:(d + 1) * 128, NI:2 * NI])
                    mk_t[d] = mk

                def stage_h(d):
                    xk = stream.tile([128, 256, KB], bf16, tag="xk")
                    nc.gpsimd.dma_start(
                        xk[:].rearrange("p j k -> p (j k)"),
                        xT_d[:, d * 256 * KB:(d + 1) * 256 * KB])
                    ph = php.tile([128, 512], f32, tag="ph")
                    for g in range(2):
                        for k in range(KB):
                            nc.tensor.matmul(
                                ph[:, g * D:(g + 1) * D],
                                lhsT=xk[:, g * 128:(g + 1) * 128, k],
                                rhs=Wh_sb[:, k, :],
                                start=(g == 0 and k == 0),
                                stop=(g == 1 and k == KB - 1),
                                skip_group_check=True)
                    ph_t[d % 2] = ph

                def stage_prep(d):
                    ph = ph_t[d % 2]
                    mk = mk_t[d]
                    a, bs = aArr[d], bsArr[d]
                    rhsA = stream.tile([128, 2, AUG], fp8, tag="rhsA")
                    rhsB = stream.tile([128, 2, AUG], fp8, tag="rhsB")
                    for g in range(2):
                        nc.scalar.activation(
                            rhsA[:, g, 0:D], ph[:, g * D:(g + 1) * D],
                            Act.Copy)
                        nc.scalar.activation(
                            rhsB[:, g, 0:D], ph[:, g * D:(g + 1) * D],
                            Act.Copy, scale=PJs[:, 2 * d + g, 1:2])
                    nc.vector.tensor_copy(rhsA[:, :, D:AUG],
                                          UVs[:, 2 * d:2 * d + 2, 0:1])
                    nc.vector.tensor_copy(rhsB[:, :, D:AUG],
                                          UVs[:, 2 * d:2 * d + 2, 1:2])
                    rhsA_t[d], rhsB_t[d] = rhsA, rhsB
                    if bs > a:
                        lo, w = a * 128, (bs - a) * 128
                        mkv = mk[:].rearrange("p (g i) -> p g i", g=2)
                        m16 = pp.tile([128, 2, maxw], bf16, tag="m16")
                        nc.vector.tensor_copy(m16[:, :, 0:w],
                                              mkv[:, :, lo:lo + w])
                        pa = pp.tile([128, 2, maxw], bf16, tag="pa")
                        for g in range(2):
                            nc.vector.tensor_scalar(
                                pa[:, g, 0:w], in0=E21[:, lo:lo + w],
                                scalar1=PJs[:, 2 * d + g, 2:3], scalar2=1.0,
                                op0=Alu.mult, op1=Alu.max)
                        pm = pp.tile([128, 2, maxw], fp8, tag="pm")
                        nc.vector.tensor_tensor(pm[:, :, 0:w], pa[:, :, 0:w],
                                                m16[:, :, 0:w], op=Alu.mult)
                        pm_t[d] = pm

                def stage_attn(d):
                    mk, pm = mk_t[d], pm_t[d]
                    rhsA, rhsB = rhsA_t[d], rhsB_t[d]
                    mkv = mk[:].rearrange("p (g i) -> p g i", g=2)
                    a, bs = aArr[d], bsArr[d]
                    first, last = (d == 0), (d == DB - 1)
                    order = ([t for t in range(TI) if not a <= t < bs]
                             + list(range(a, bs)))
                    if first:
                        order = list(range(TI))
                    for t in order:
                        if t < a:
                            lhsT = mkv[:, :, t * 128:(t + 1) * 128]
                            rhs = rhsA[:]
                        elif t < bs:
                            lhsT = pm[:, :, (t - a) * 128:(t - a + 1) * 128]
                            rhs = rhsB[:]
                        else:
                            lhsT = mkv[:, :, t * 128:(t + 1) * 128]
                            rhs = rhsB[:]
                        nc.tensor.matmul(
                            accv(t), lhsT=lhsT, rhs=rhs,
                            start=(first and t % 3 == 0), stop=last,
                            perf_mode=mybir.MatmulPerfMode.DoubleRow,
                            skip_group_check=True)

                issue_mask(0)
                issue_mask(1)
                nc.scalar.dma_start(PJs[:].rearrange("p b k -> p (b k)"),
                                    PJ_d[:])
                nc.scalar.dma_start(UVs[:].rearrange("p b k -> p (b k)"),
                                    UV_d[:])
                nc.sync.dma_start(E21[:], E21_d[:])
                stage_h(0)
                stage_prep(0)
                for d in range(1, DB):
                    if d + 1 < DB:
                        issue_mask(d + 1)
                    stage_h(d)
                    stage_attn(d - 1)
                    stage_prep(d)
                stage_attn(DB - 1)

                with tc.tile_pool(name="tail", bufs=2) as tail_pool:
                    for bk in range(6):
                        nt = 3 if bk < 5 else 1
                        W3 = nt * D
                        bank = accb[bk]
                        rinv = tail_pool.tile([128, 3], f32, tag="rinv")
                        bv = bank[:, 0:nt * 130].rearrange(
                            "p (t c) -> p t c", c=130)
                        nc.vector.reciprocal(rinv[:, 0:nt], bv[:, :, D])
                        osb = tail_pool.tile([128, 3, D], bf16, tag="osb")
                        nc.vector.tensor_tensor(
                            osb[:, 0:nt, :], bv[:, :, 0:D],
                            rinv[:, 0:nt].unsqueeze(2).broadcast_to(
                                [128, nt, D]),
                            op=Alu.mult)
                        ov = osb[:].rearrange("p t c -> p (t c)")[:, 0:W3]
                        ez = tail_pool.tile([128, 3 * D], bf16, tag="ez")
                        nc.scalar.activation(ez[:, 0:W3], ov, Act.Exp)
                        rm1 = tail_pool.tile([128, 3 * D], bf16, tag="rm1")
                        nc.vector.tensor_scalar(rm1[:, 0:W3], in0=ov,
                                                scalar1=0.0, scalar2=-1.0,
                                                op0=Alu.max, op1=Alu.add)
                        oo = tail_pool.tile([128, 3 * D], bf16, tag="oo")
                        nc.vector.scalar_tensor_tensor(
                            oo[:, 0:W3], in0=ez[:, 0:W3], scalar=1.0,
                            in1=rm1[:, 0:W3], op0=Alu.min, op1=Alu.add)
                        nc.scalar.dma_start(
                            out_d[bk * 384:bk * 384 + W3, :]
                            .rearrange("(t p) d -> p t d", p=128),
                            oo[:, 0:W3].rearrange("p (t c) -> p t c", c=D))

    nc.compile()
    return nc


def _layout(x, adj, W, a):
    """Host: scores, sorts, staircase union, per-core params."""
    import ml_dtypes
    bfdt = ml_dtypes.bfloat16
    f8 = ml_dtypes.float8_e4m3fn

    x = np.ascontiguousarray(np.asarray(x, np.float32))
    adj = np.asarray(adj)
    W = np.ascontiguousarray(np.asarray(W, np.float32))
    a = np.asarray(a, np.float32)

    a1, a2 = a[:D, 0], a[D:, 0]
    WA = np.zeros((F, 2 * H), np.float32)
    for h in range(H):
        WA[:, h] = W[:, h * D:(h + 1) * D] @ a1
        WA[:, H + h] = W[:, h * D:(h + 1) * D] @ a2
    S = x @ WA
    SI, SJ = S[:, :H], S[:, H:]

    xT16 = np.ascontiguousarray(x.T.astype(bfdt))
    adjT = np.ascontiguousarray(adj.T.astype(f8))

    heads = []
    for h in range(H):
        pj = np.argsort(SJ[:, h], kind="stable")
        pi = np.argsort(SI[:, h], kind="stable")
        sjs = SJ[pj, h]
        heads.append({
            "pj": pj, "pi": pi, "sjs": sjs,
            "xTp": np.ascontiguousarray(
                (xT16[:, pj].astype(np.float32)
                 * np.exp(0.2 * sjs - 3.0)[None, :]).astype(bfdt)
                .reshape(KB, 128, N).transpose(1, 2, 0)
                .reshape(128, N * KB)),
            "maskh": np.ascontiguousarray(adjT[pj]),
            "Wh": np.ascontiguousarray(W[:, h * D:(h + 1) * D].astype(bfdt)),
            "PJ": np.ascontiguousarray(np.stack(
                [np.exp(0.2 * sjs - 3.0), np.exp(0.8 * sjs - 3.0),
                 np.exp(-0.8 * sjs)], axis=1).astype(np.float32)
                .reshape(JB, 128, 3).transpose(1, 0, 2)
                .reshape(128, JB * 3)),
            "UV": np.ascontiguousarray(np.stack(
                [np.exp(0.2 * sjs - 3.0), np.exp(sjs - 6.0)],
                axis=1).astype(bfdt)
                .reshape(JB, 128, 2).transpose(1, 0, 2)
                .reshape(128, JB * 2)),
        })

    # staircase per core, then union
    sjlo = np.array([heads[h]["sjs"][b * 128] for h in range(H)
                     for b in [0]])  # placeholder
    aC = np.zeros((M, JB), np.int64)
    bC = np.zeros((M, JB), np.int64)
    cores = []
    for c in range(M):
        h, half = c // 2, c % 2
        hd = heads[h]
        ic = hd["pi"][half::2]
        sis = SI[ic, h]                       # ascending
        simin = sis[0::128][:TI]
        simax = sis[127::128][:TI]
        sjs = hd["sjs"]
        for b in range(JB):
            lo, hi = sjs[b * 128], sjs[b * 128 + 127]
            aC[c, b] = int(np.sum(-simax >= hi))
            bC[c, b] = TI - int(np.sum(lo >= -simin))
        cores.append({"ic": ic, "h": h, "hd": hd, "sis": sis})

    aU = aC.min(axis=0)
    bU = bC.max(axis=0)
    aU = np.minimum(aU, bU)
    # DR-step (256-j) classification: union of block pairs
    aU = np.minimum(aU[0::2], aU[1::2])
    bU = np.maximum(bU[0::2], bU[1::2])
    assert np.all(aU <= bU) and np.all(bU <= TI)
    assert np.all(bU - aU) <= 3 or True
    key = (tuple(int(v) for v in aU), tuple(int(v) for v in bU))

    in_maps = []
    for c in range(M):
        co = cores[c]
        hd = co["hd"]
        ic = co["ic"]
        mask = hd["maskh"][:, ic]             # fp8 [N, NI]
        E21f = np.exp(-0.8 * co["sis"]).astype(np.float32)
        # hybrid mask: pure-A columns carry mask*E21 (fp8), rest raw mask
        mh = np.array(mask)
        mf = mask.astype(np.float32)
        for d in range(JB // 2):
            aw = int(aU[d]) * 128
            rows = slice(d * 256, (d + 1) * 256)
            if aw:
                mh[rows, :aw] = (mf[rows, :aw] * E21f[None, :aw]
                                 * np.float32(np.exp(-3.0))).astype(f8)
        # DR row pairing: [N/2, 2*NI]; row d*128+p = (j=d*256+p, j=d*256+128+p)
        mh = mh.reshape(JB // 2, 2, 128, NI).transpose(0, 2, 1, 3).reshape(
            N // 2, 2 * NI)
        # mixed-band bf16 mask, DR-paired, per-group stride 384
        mm = np.zeros((JB // 2, 2, 128, 384), np.float32)
        for dd in range(JB // 2):
            aw = int(aU[dd]) * 128
            w = (int(bU[dd]) - int(aU[dd])) * 128
            if w:
                mm[dd, :, :, 0:w] = mf[dd * 256:(dd + 1) * 256, aw:aw + w]\
                    .reshape(2, 128, w)
        mm = mm.transpose(0, 2, 1, 3).reshape(N // 2, 2 * 384).astype(bfdt)
        in_maps.append({
            "xT": hd["xTp"],
            "Wh": hd["Wh"],
            "maskH": np.ascontiguousarray(mh),
            "E21": np.ascontiguousarray(np.broadcast_to(E21f[None, :].astype(bfdt), (128, NI))),
            "PJ": hd["PJ"],
            "maskM": np.ascontiguousarray(mm),
            "UV": hd["UV"],
        })
    scat = [(cores[c]["ic"], cores[c]["h"]) for c in range(M)]
    return in_maps, key, scat


def _host_prep(x, adj, W, a):
    in_maps, key, scat = _layout(x, adj, W, a)
    _CACHE["key"] = key
    _CACHE["scat"] = scat
    return in_maps


def kernel(x, adj, W, a):
    from concourse.bass_utils import run_bass_kernel_spmd

    in_maps, key, scat = _layout(x, adj, W, a)
    if _CACHE.get("nc_key") != key:
        _CACHE["nc"] = _build_nc(list(key[0]), list(key[1]))
        _CACHE["nc_key"] = key
    nc = _CACHE["nc"]

    res = run_bass_kernel_spmd(nc, in_maps, list(range(M)))
    out = np.empty((N, F), np.float32)
    for c in range(M):
        ic, h = scat[c]
        out[ic, h * D:(h + 1) * D] = np.asarray(res.results[c]["out"],
                                                np.float32)
    return out


if __name__ == "__main__":
    print("kernel module ok")
